# revision 1
# baseline (speedup 1.0000x reference)
"""GAT (3-layer) kernel for Trainium2, 8 NeuronCores.

Sharding (per hint): nodes partitioned across 8 cores. The encoder
matmul h = x @ enc_W runs on-device as a Bass/Tile SPMD kernel with
x row-sharded 8 ways (x is fed pre-transposed so the contraction dim
sits on SBUF partitions); weights replicated. The irregular
segment-softmax message passing runs on host with sorted-edge
reduceat segment ops (every dst segment is non-empty thanks to
self-loops).
"""

import numpy as np

N, E, D = 100000, 1600000, 128
L = 3
NCORES = 8
PER = N // NCORES  # 12500
CHUNK = 500        # 25 chunks of 500 node-columns per core
EPS = 1e-5
NEG_SLOPE = 0.2

_BASS_CACHE = {}


def _build_encoder_kernel():
    if "nc" in _BASS_CACHE:
        return _BASS_CACHE["nc"]
    import concourse.bass as bass
    import concourse.tile as tile
    from concourse import mybir

    nc = bass.Bass()
    xT = nc.declare_dram_parameter("xT", [D, PER], mybir.dt.float32, isOutput=False)
    W = nc.declare_dram_parameter("W", [D, D], mybir.dt.float32, isOutput=False)
    hT = nc.declare_dram_parameter("hT", [D, PER], mybir.dt.float32, isOutput=True)

    with tile.TileContext(nc) as tc:
        with (
            tc.tile_pool(name="wpool", bufs=1) as wpool,
            tc.tile_pool(name="inpool", bufs=3) as inpool,
            tc.tile_pool(name="outpool", bufs=25) as outpool,
            tc.tile_pool(name="psum", bufs=2, space=bass.MemorySpace.PSUM) as psum,
        ):
            wt0 = wpool.tile([D, D], mybir.dt.float32, tag="w0")
            nc.gpsimd.dma_start(wt0[:], W[:])
            wt = wpool.tile([D, D], mybir.dt.float32, tag="w1")
            # bounce DMA'd tiles through the vector engine so the PE
            # matmul waits on one compute sem, not N DMA-queue sems
            nc.vector.tensor_copy(wt[:], wt0[:])
            for i in range(PER // CHUNK):
                xt0 = inpool.tile([D, CHUNK], mybir.dt.float32, tag="x0")
                nc.gpsimd.dma_start(xt0[:], xT[:, i * CHUNK:(i + 1) * CHUNK])
                xt = inpool.tile([D, CHUNK], mybir.dt.float32, tag="x1")
                nc.vector.tensor_copy(xt[:], xt0[:])
                acc = psum.tile([D, CHUNK], mybir.dt.float32)
                # acc = W.T @ x.T-chunk = (x-chunk @ W).T
                nc.tensor.matmul(acc[:], wt[:], xt[:])
                ot = outpool.tile([D, CHUNK], mybir.dt.float32)
                nc.vector.tensor_copy(ot[:], acc[:])
                nc.gpsimd.dma_start(hT[:, i * CHUNK:(i + 1) * CHUNK], ot[:])

    _BASS_CACHE["nc"] = nc
    return nc


def _encode_device(x, enc_W):
    from concourse.bass_utils import run_bass_kernel_spmd

    nc = _build_encoder_kernel()
    xT = np.ascontiguousarray(x.T.astype(np.float32, copy=False))
    w = np.ascontiguousarray(enc_W.astype(np.float32, copy=False))
    in_maps = [
        {"xT": np.ascontiguousarray(xT[:, i * PER:(i + 1) * PER]), "W": w}
        for i in range(NCORES)
    ]
    res = run_bass_kernel_spmd(nc, in_maps, list(range(NCORES))).results
    return np.concatenate(
        [np.ascontiguousarray(res[i]["hT"].T) for i in range(NCORES)], axis=0
    )


def kernel(x, edge_index, enc_W, enc_b, Wg, a_src, a_dst, bg, ln_w, ln_b,
           dec_W, dec_b):
    x = np.asarray(x, dtype=np.float32)
    enc_W = np.asarray(enc_W, dtype=np.float32)
    enc_b = np.asarray(enc_b, dtype=np.float32)
    Wg = np.asarray(Wg, dtype=np.float32)
    a_src = np.asarray(a_src, dtype=np.float32)
    a_dst = np.asarray(a_dst, dtype=np.float32)
    bg = np.asarray(bg, dtype=np.float32)
    ln_w = np.asarray(ln_w, dtype=np.float32)
    ln_b = np.asarray(ln_b, dtype=np.float32)
    dec_W = np.asarray(dec_W, dtype=np.float32)
    dec_b = np.asarray(dec_b, dtype=np.float32)
    edge_index = np.asarray(edge_index)

    try:
        h = _encode_device(x, enc_W)
    except Exception:
        h = x @ enc_W
    h = (h + enc_b).astype(np.float32)

    loop = np.arange(N, dtype=edge_index.dtype)
    src = np.concatenate([edge_index[0], loop])
    dst = np.concatenate([edge_index[1], loop])
    perm = np.argsort(dst, kind="stable")
    src_s = src[perm]
    dst_s = dst[perm]
    # every dst has >=1 incident edge (self-loops), so all segments non-empty
    starts = np.searchsorted(dst_s, np.arange(N, dtype=dst_s.dtype), "left")

    for i in range(L):
        h_in = h
        hw = (h @ Wg[i]).astype(np.float32)
        al_s = hw @ a_src[i]
        al_d = hw @ a_dst[i]
        e = al_s[src_s] + al_d[dst_s]
        e = np.where(e >= 0, e, np.float32(NEG_SLOPE) * e).astype(np.float32)
        m = np.maximum.reduceat(e, starts)
        ex = np.exp(e - m[dst_s], dtype=np.float32)
        denom = np.add.reduceat(ex, starts)
        alpha = (ex / denom[dst_s]).astype(np.float32)
        msg = hw[src_s]
        msg *= alpha[:, None]
        out = np.add.reduceat(msg, starts, axis=0).astype(np.float32)
        del msg
        out = out + bg[i]
        mean = np.float32(out.mean(dtype=np.float64))
        var = np.float32(np.mean((out - mean) ** 2, dtype=np.float64))
        hn = ln_w[i] * (out - mean) * np.float32(1.0 / np.sqrt(var + EPS)) + ln_b[i]
        h = (np.maximum(hn, 0) + h_in).astype(np.float32)

    z = (h @ dec_W + dec_b).astype(np.float32)
    sig = 1.0 / (1.0 + np.exp(-z, dtype=np.float32))
    return sig.sum(axis=0, dtype=np.float32).astype(np.float32)



# revision 21
# speedup vs baseline: 522.1747x; 522.1747x over previous
"""3-layer GAT on Trainium2, 8 NeuronCores, full computation on device.

Sharding: nodes partitioned by dst ownership (nsh=12500/core). Edges are
dst-sorted and grouped into 128-edge tiles covering a FIXED range of K
consecutive dst nodes (K chosen at runtime so no K-node group exceeds
128 edges; K=5 for the reference graph). Fixed K makes every tile's
output rows linear in the tile index, so segment sums land in outbuf via
plain DMA writes — the only indirect DMA is the per-tile [128,1]-offset
row gather of hw_aug[src] (the hardware-validated DGE pattern).

Per layer each core: (1) recomputes full hw_aug = [h @ Wg | as] rows plus
an ad table (replicated compute instead of communicating hw), (2) edge
loop over tile groups: gather hw_aug[src] rows (as rides along as column
128), load the group's ad values with a linear partition_id-based slice,
build the per-(edge,slot) weight matrix M = exp(leaky_relu(as_e + ad_s))
* [slot(e)==s] with a handful of batched [128, G*K] vector ops, then per
tile two PE matmuls produce the K x (D+1) segment sums (features +
softmax denominator), (3) graph-LayerNorm via AllReduce of (sum, sumsq),
relu, residual, (4) AllGather of updated h shards. Softmax is max-free
(e stays in [-0.6, 1.8] at this data scale; exp cannot overflow).
Decoder computes per-core partial sigmoid-sums; host adds the 8 partials.
"""

import numpy as np

NC = 8
P = 128
D = 128
DA = 132          # hw_aug row: 128 feats + as + 3 pad (16B-aligned row)
L = 3
EPS = 1e-5
NEG = 0.2
CH = 512          # pre-pass node chunk
G = 16            # edge tiles per loop group

_CACHE = {}


def _choose_k(counts, nsh):
    for k in (6, 5, 4, 3, 2, 1):
        ok = True
        for c in range(NC):
            cc = counts[c * nsh:(c + 1) * nsh]
            gs = np.add.reduceat(cc, np.arange(0, nsh, k))
            if gs.max() > P:
                ok = False
                break
        if ok:
            return k
    raise AssertionError("node with degree > 128")


def _prep(src, dst, n_full, nsh):
    """Edge preprocessing -> per-core wrapped (src, slot) arrays + K."""
    perm = np.argsort(dst, kind="stable")
    src_s = np.ascontiguousarray(src[perm]).astype(np.int64)
    dst_s = np.ascontiguousarray(dst[perm]).astype(np.int64)
    starts = np.searchsorted(dst_s, np.arange(n_full + 1, dtype=np.int64), "left")
    counts = np.diff(starts)
    k = _choose_k(counts, nsh)
    ngrp = (nsh + k - 1) // k
    t_pad = ((ngrp + G - 1) // G) * G
    cores = []
    for c in range(NC):
        lo = c * nsh
        e0, e1 = starts[lo], starts[lo + nsh]
        ne = e1 - e0
        dstloc = dst_s[e0:e1] - lo
        g = dstloc // k
        s = (dstloc % k).astype(np.float32)
        grp_first = np.searchsorted(g, np.arange(ngrp))
        epos = g * P + (np.arange(ne) - grp_first[g])
        srcA = np.zeros(t_pad * P, np.int32)
        slotA = np.full(t_pad * P, float(k), np.float32)   # sentinel slot
        srcA[epos] = src_s[e0:e1]
        slotA[epos] = s
        cores.append({
            "srcA": np.ascontiguousarray(srcA.reshape(t_pad, P).T),
            "slotA": np.ascontiguousarray(slotA.reshape(t_pad, P).T),
        })
    return cores, t_pad, k


def _build(nsh, t_pad, fr, k, ncores):
    import concourse.bacc as bacc
    import concourse.tile as tile
    from concourse import mybir
    from concourse.bass import IndirectOffsetOnAxis, ds
    from concourse.masks import make_identity

    f32 = mybir.dt.float32
    i32 = mybir.dt.int32
    AT = mybir.ActivationFunctionType
    OP = mybir.AluOpType

    n_full = nsh * ncores
    n_pad = ((n_full + CH - 1) // CH) * CH
    while (ncores - 1) * nsh + k * t_pad + P > n_pad:
        n_pad += CH
    gk = G * k
    nrows = k * t_pad
    nd_inv = 1.0 / (float(n_full) * D)

    nc = bacc.Bacc()
    xs = nc.declare_dram_parameter("xs", [nsh, D], f32, isOutput=False)
    encW = nc.declare_dram_parameter("encW", [D, D], f32, isOutput=False)
    encb = nc.declare_dram_parameter("encb", [P, D], f32, isOutput=False)
    WgP = nc.declare_dram_parameter("WgP", [L, D, D], f32, isOutput=False)
    a2P = nc.declare_dram_parameter("a2P", [L, D, 2], f32, isOutput=False)
    bgP = nc.declare_dram_parameter("bgP", [L, P, D], f32, isOutput=False)
    lnwP = nc.declare_dram_parameter("lnwP", [L, P, D], f32, isOutput=False)
    lnbP = nc.declare_dram_parameter("lnbP", [L, P, D], f32, isOutput=False)
    decW = nc.declare_dram_parameter("decW", [D, 1], f32, isOutput=False)
    decb = nc.declare_dram_parameter("decb", [1, 1], f32, isOutput=False)
    srcA = nc.declare_dram_parameter("srcA", [P, t_pad], i32, isOutput=False)
    slotA = nc.declare_dram_parameter("slotA", [P, t_pad], f32, isOutput=False)
    outp = nc.declare_dram_parameter("outp", [1, 1], f32, isOutput=True)

    h_full = nc.dram_tensor("h_full", [n_pad, D], f32, addr_space="Shared")
    cc_in = nc.dram_tensor("cc_in", [nsh, D], f32)
    st_in = nc.dram_tensor("st_in", [1, 2], f32)
    st_out = nc.dram_tensor("st_out", [1, 2], f32, addr_space="Shared")
    hw_aug = nc.dram_tensor("hw_aug", [n_pad, DA], f32)
    ad_dram = nc.dram_tensor("ad_dram", [1, n_pad], f32)
    outbuf = nc.dram_tensor("outbuf", [nrows + P, D + 1], f32)
    conv_dram = nc.dram_tensor("conv_dram", [nsh, D], f32)
    hsh_dram = nc.dram_tensor("hsh_dram", [nsh, D], f32)
    rg = [list(range(ncores))]

    with tile.TileContext(nc) as tc:
        with (
            tc.tile_pool(name="cst", bufs=1) as cst,
            tc.tile_pool(name="wts", bufs=1) as wts,
            tc.tile_pool(name="sml", bufs=2) as sml,
        ):
            ident = cst.tile([P, P], f32)
            make_identity(nc, ident[:])
            iotaK = cst.tile([P, gk], f32)
            nc.gpsimd.iota(iotaK[:], pattern=[[0, G], [1, k]], base=0,
                           channel_multiplier=0,
                           allow_small_or_imprecise_dtypes=True)
            ones_col = cst.tile([P, 1], f32)
            nc.vector.memset(ones_col[:], 1.0)
            one_row = cst.tile([1, P], f32)
            nc.vector.memset(one_row[:], 1.0)
            if n_pad > n_full:
                zrow = cst.tile([P, D], f32)
                nc.vector.memset(zrow[:], 0.0)
                r = n_full
                while r < n_pad:
                    q = min(P, n_pad - r)
                    nc.sync.dma_start(h_full[r:r + q, :], zrow[:q, :])
                    r += q

            encW_sb = wts.tile([D, D], f32)
            nc.sync.dma_start(encW_sb[:], encW[:])
            encb_sb = wts.tile([P, D], f32)
            nc.sync.dma_start(encb_sb[:], encb[:])
            decW_sb = wts.tile([D, 1], f32)
            nc.sync.dma_start(decW_sb[:], decW[:])
            decb_sb = wts.tile([1, 1], f32)
            nc.sync.dma_start(decb_sb[:], decb[:])

            # ---------------- encoder ----------------
            with (
                tc.tile_pool(name="ep", bufs=3) as ep,
                tc.tile_pool(name="eps", bufs=2, space="PSUM") as eps,
            ):
                def enc_body(iv):
                    xt = ep.tile([fr, D], f32, tag="xt")
                    nc.sync.dma_start(xt[:], xs[ds(iv, fr), :])
                    pT = eps.tile([D, fr], f32, tag="pT")
                    nc.tensor.transpose(pT[:], xt[:], ident[:fr, :fr])
                    xT = ep.tile([D, fr], f32, tag="xT")
                    nc.vector.tensor_copy(xT[:], pT[:])
                    ph = eps.tile([fr, D], f32, tag="ph")
                    nc.tensor.matmul(ph[:], xT[:], encW_sb[:], start=True, stop=True)
                    h0 = ep.tile([fr, D], f32, tag="h0")
                    nc.vector.tensor_tensor(out=h0[:], in0=ph[:], in1=encb_sb[:fr, :], op=OP.add)
                    nc.sync.dma_start(cc_in[ds(iv, fr), :], h0[:])
                    nc.sync.dma_start(hsh_dram[ds(iv, fr), :], h0[:])
                tc.For_i_unrolled(0, nsh, fr, enc_body, max_unroll=4)

            nc.gpsimd.collective_compute(
                "AllGather", OP.bypass, replica_groups=rg,
                ins=[cc_in[:, :]], outs=[h_full[0:n_full, :]])

            # ---------------- layers ----------------
            for l in range(L):
                with tc.tile_pool(name=f"lw{l}", bufs=1) as lw:
                    Wg_sb = lw.tile([D, D], f32)
                    nc.sync.dma_start(Wg_sb[:], WgP[l, :, :])
                    a2_sb = lw.tile([D, 2], f32)
                    nc.sync.dma_start(a2_sb[:], a2P[l, :, :])
                    bg_sb = lw.tile([P, D], f32)
                    nc.sync.dma_start(bg_sb[:], bgP[l, :, :])
                    lnw_sb = lw.tile([P, D], f32)
                    nc.sync.dma_start(lnw_sb[:], lnwP[l, :, :])
                    lnb_sb = lw.tile([P, D], f32)
                    nc.sync.dma_start(lnb_sb[:], lnbP[l, :, :])

                    # ---- pre-pass: hw_aug rows + ad table, all nodes ----
                    with (
                        tc.tile_pool(name="pp", bufs=3) as pp,
                        tc.tile_pool(name="ppsA", bufs=2, space="PSUM") as ppsA,
                        tc.tile_pool(name="ppsB", bufs=2, space="PSUM") as ppsB,
                        tc.tile_pool(name="ppsC", bufs=1, space="PSUM") as ppsC,
                    ):
                        def pre_body(iv):
                            hch = pp.tile([P, 4, D], f32, tag="hch")
                            nc.sync.dma_start(
                                hch[:], h_full[ds(iv, CH), :].rearrange(
                                    "(a p) d -> p a d", p=P))
                            pT = ppsA.tile([P, CH], f32, tag="pT")
                            for b in range(4):
                                nc.tensor.transpose(
                                    pT[:, b * P:(b + 1) * P], hch[:, b, :], ident[:])
                            hT = pp.tile([P, CH], f32, tag="hT")
                            nc.vector.tensor_copy(hT[:], pT[:])
                            phw = ppsB.tile([P, CH], f32, tag="phw")
                            nc.tensor.matmul(phw[:], Wg_sb[:], hT[:], start=True, stop=True)
                            hwT = pp.tile([P, CH], f32, tag="hwT")
                            nc.vector.tensor_copy(hwT[:], phw[:])
                            pas = ppsC.tile([2, CH], f32, tag="pas")
                            nc.tensor.matmul(pas[:], a2_sb[:], hwT[:], start=True, stop=True)
                            phw2 = ppsA.tile([P, CH], f32, tag="phw2")
                            for b in range(4):
                                nc.tensor.transpose(
                                    phw2[:, b * P:(b + 1) * P],
                                    hwT[:, b * P:(b + 1) * P], ident[:])
                            asv = pp.tile([2, CH], f32, tag="asv")
                            nc.vector.tensor_copy(asv[:], pas[:])
                            pasT = ppsC.tile([P, 8], f32, tag="pasT")
                            for b in range(4):
                                nc.tensor.transpose(
                                    pasT[:, b * 2:(b + 1) * 2],
                                    asv[:, b * P:(b + 1) * P], ident[:2, :2])
                            stg = pp.tile([P, 4, DA], f32, tag="stg")
                            nc.vector.tensor_copy(
                                stg[:, :, 0:D], phw2[:].rearrange("p (a d) -> p a d", a=4))
                            nc.scalar.activation(
                                stg[:, :, D:D + 2],
                                pasT[:].rearrange("p (a t) -> p a t", a=4),
                                AT.Copy)
                            nc.vector.memset(stg[:, :, D + 2:DA], 0.0)
                            nc.sync.dma_start(
                                hw_aug[ds(iv, CH), :].rearrange("(a p) d -> p a d", p=P),
                                stg[:])
                            nc.sync.dma_start(ad_dram[0:1, ds(iv, CH)], asv[1:2, :])
                        tc.For_i_unrolled(0, n_pad, CH, pre_body, max_unroll=4)

                    # ---- edge loop ----
                    with (
                        tc.tile_pool(name="eb", bufs=3) as eb,
                        tc.tile_pool(name="ebR", bufs=2, space="PSUM") as ebR,
                        tc.tile_pool(name="ebo", bufs=4, space="PSUM") as ebo,
                    ):
                        pid_s = nc.sync.partition_id()
                        with tc.For_i(0, t_pad, G) as i0:
                            adr = eb.tile([1, P], f32, tag="adr")
                            nc.sync.dma_start(
                                adr[:], ad_dram[0:1, ds(pid_s * nsh + i0 * k, P)])
                            Rp = ebR.tile([P, gk], f32, tag="Rp")
                            nc.tensor.matmul(Rp[:], one_row[:], adr[:, 0:gk],
                                             start=True, stop=True)
                            six = eb.tile([P, G], i32, tag="six")
                            nc.sync.dma_start(six[:], srcA[:, ds(i0, G)])
                            slb = eb.tile([P, G, 1], f32, tag="slb")
                            nc.sync.dma_start(slb[:, :, 0], slotA[:, ds(i0, G)])
                            msgb = eb.tile([P, G, DA], f32, tag="msgb")
                            for j in range(G):
                                nc.gpsimd.indirect_dma_start(
                                    out=msgb[:, j, :], out_offset=None, in_=hw_aug[:],
                                    in_offset=IndirectOffsetOnAxis(
                                        ap=six[:, j:j + 1], axis=0))
                            Bt = eb.tile([P, G, k], f32, tag="Bt")
                            nc.vector.tensor_tensor(
                                out=Bt[:],
                                in0=Rp[:].rearrange("p (g s) -> p g s", g=G),
                                in1=msgb[:, :, D:D + 1].to_broadcast([P, G, k]),
                                op=OP.add)
                            lr = eb.tile([P, G, k], f32, tag="lr")
                            nc.vector.tensor_scalar_mul(lr[:], Bt[:], NEG)
                            nc.vector.tensor_tensor(out=lr[:], in0=lr[:], in1=Bt[:], op=OP.max)
                            M0 = eb.tile([P, G, k], f32, tag="M0")
                            nc.scalar.activation(M0[:], lr[:], AT.Exp)
                            Sel = eb.tile([P, G, k], f32, tag="Sel")
                            nc.vector.tensor_tensor(
                                out=Sel[:],
                                in0=slb[:].to_broadcast([P, G, k]),
                                in1=iotaK[:].rearrange("p (g s) -> p g s", g=G),
                                op=OP.is_equal)
                            nc.vector.tensor_tensor(out=Sel[:], in0=Sel[:], in1=M0[:], op=OP.mult)
                            for j in range(G):
                                po = ebo.tile([k, D + 1], f32, tag="po")
                                nc.tensor.matmul(po[:, 0:D], Sel[:, j, :],
                                                 msgb[:, j, 0:D], start=True, stop=True)
                                nc.tensor.matmul(po[:, D:D + 1], Sel[:, j, :],
                                                 ones_col[:], start=True, stop=True)
                                st = eb.tile([k, D + 1], f32, tag="st")
                                if j % 2 == 0:
                                    nc.vector.tensor_copy(st[:], po[:])
                                else:
                                    nc.scalar.activation(st[:], po[:], AT.Copy)
                                nc.sync.dma_start(
                                    outbuf[ds(i0 * k + j * k, k), :], st[:])

                    # ---- finalize: conv = msg/denom + bg ; stats ----
                    with (
                        tc.tile_pool(name="fp", bufs=3) as fp,
                        tc.tile_pool(name="facc", bufs=1) as facc,
                        tc.tile_pool(name="fps", bufs=2, space="PSUM") as fps,
                    ):
                        acc = facc.tile([fr, 2], f32)
                        nc.vector.memset(acc[:], 0.0)

                        def fin_body(iv):
                            ob = fp.tile([fr, D + 1], f32, tag="ob")
                            nc.sync.dma_start(ob[:], outbuf[ds(iv, fr), :])
                            rcp = fp.tile([fr, 1], f32, tag="rcp")
                            nc.vector.reciprocal(rcp[:], ob[:, D:D + 1])
                            cv = fp.tile([fr, D], f32, tag="cv")
                            nc.vector.tensor_tensor(
                                out=cv[:], in0=ob[:, 0:D],
                                in1=rcp[:].to_broadcast([fr, D]), op=OP.mult)
                            nc.vector.tensor_tensor(
                                out=cv[:], in0=cv[:], in1=bg_sb[:fr, :], op=OP.add)
                            s1 = fp.tile([fr, 1], f32, tag="s1")
                            nc.vector.tensor_reduce(
                                out=s1[:], in_=cv[:], axis=mybir.AxisListType.X, op=OP.add)
                            sqv = fp.tile([fr, D], f32, tag="sqv")
                            s2 = fp.tile([fr, 1], f32, tag="s2")
                            nc.scalar.activation(sqv[:], cv[:], AT.Square, accum_out=s2[:])
                            nc.vector.tensor_tensor(
                                out=acc[:, 0:1], in0=acc[:, 0:1], in1=s1[:], op=OP.add)
                            nc.vector.tensor_tensor(
                                out=acc[:, 1:2], in0=acc[:, 1:2], in1=s2[:], op=OP.add)
                            nc.sync.dma_start(conv_dram[ds(iv, fr), :], cv[:])
                        tc.For_i_unrolled(0, nsh, fr, fin_body, max_unroll=4)

                        pst = fps.tile([1, 2], f32)
                        nc.tensor.matmul(pst[:], ones_col[:fr, :], acc[:], start=True, stop=True)
                        stt = sml.tile([1, 2], f32, tag="stt")
                        nc.vector.tensor_copy(stt[:], pst[:])
                        nc.sync.dma_start(st_in[:, :], stt[:])

                    nc.gpsimd.collective_compute(
                        "AllReduce", OP.add, replica_groups=rg,
                        ins=[st_in[:, :]], outs=[st_out[:, :]])

                    # ---- stats -> scale/shift, apply LN + relu + residual ----
                    with (
                        tc.tile_pool(name="ap", bufs=3) as apl,
                        tc.tile_pool(name="aps", bufs=2, space="PSUM") as aps,
                    ):
                        sto = sml.tile([1, 2], f32, tag="sto")
                        nc.sync.dma_start(sto[:], st_out[:, :])
                        mn = sml.tile([1, 1], f32, tag="mn")
                        nc.vector.tensor_scalar_mul(mn[:], sto[:, 0:1], nd_inv)
                        ms = sml.tile([1, 1], f32, tag="ms")
                        nc.vector.tensor_scalar_mul(ms[:], sto[:, 1:2], nd_inv)
                        m2 = sml.tile([1, 1], f32, tag="m2")
                        nc.vector.tensor_tensor(out=m2[:], in0=mn[:], in1=mn[:], op=OP.mult)
                        vr = sml.tile([1, 1], f32, tag="vr")
                        nc.vector.tensor_tensor(out=vr[:], in0=ms[:], in1=m2[:], op=OP.subtract)
                        nc.vector.tensor_scalar_add(vr[:], vr[:], EPS)
                        sd = sml.tile([1, 1], f32, tag="sd")
                        nc.scalar.activation(sd[:], vr[:], AT.Sqrt)
                        rs = sml.tile([1, 1], f32, tag="rs")
                        nc.vector.reciprocal(rs[:], sd[:])
                        nmr = sml.tile([1, 1], f32, tag="nmr")
                        nc.vector.tensor_tensor(out=nmr[:], in0=mn[:], in1=rs[:], op=OP.mult)
                        nc.vector.tensor_scalar_mul(nmr[:], nmr[:], -1.0)
                        pk = sml.tile([1, 2], f32, tag="pk")
                        nc.vector.tensor_copy(pk[:, 0:1], rs[:])
                        nc.vector.tensor_copy(pk[:, 1:2], nmr[:])
                        pbc = aps.tile([P, 2], f32)
                        nc.tensor.matmul(pbc[:], one_row[:], pk[:], start=True, stop=True)
                        bc = sml.tile([P, 2], f32, tag="bc")
                        nc.vector.tensor_copy(bc[:], pbc[:])

                        last = (l == L - 1)

                        def app_body(iv):
                            cv = apl.tile([fr, D], f32, tag="acv")
                            nc.sync.dma_start(cv[:], conv_dram[ds(iv, fr), :])
                            tt = apl.tile([fr, D], f32, tag="att")
                            nc.vector.tensor_scalar(
                                out=tt[:], in0=cv[:], scalar1=bc[:fr, 0:1],
                                scalar2=bc[:fr, 1:2], op0=OP.mult, op1=OP.add)
                            nc.vector.tensor_tensor(
                                out=tt[:], in0=tt[:], in1=lnw_sb[:fr, :], op=OP.mult)
                            nc.vector.tensor_tensor(
                                out=tt[:], in0=tt[:], in1=lnb_sb[:fr, :], op=OP.add)
                            nc.vector.tensor_scalar_max(tt[:], tt[:], 0.0)
                            hin = apl.tile([fr, D], f32, tag="hin")
                            nc.sync.dma_start(hin[:], hsh_dram[ds(iv, fr), :])
                            nc.vector.tensor_tensor(
                                out=tt[:], in0=tt[:], in1=hin[:], op=OP.add)
                            nc.sync.dma_start(hsh_dram[ds(iv, fr), :], tt[:])
                            if not last:
                                nc.sync.dma_start(cc_in[ds(iv, fr), :], tt[:])
                        tc.For_i_unrolled(0, nsh, fr, app_body, max_unroll=4)

                    if l < L - 1:
                        nc.gpsimd.collective_compute(
                            "AllGather", OP.bypass, replica_groups=rg,
                            ins=[cc_in[:, :]], outs=[h_full[0:n_full, :]])

            # ---------------- decoder ----------------
            with (
                tc.tile_pool(name="dp", bufs=3) as dp,
                tc.tile_pool(name="dacc", bufs=1) as dac,
                tc.tile_pool(name="dps", bufs=2, space="PSUM") as dps,
            ):
                dacc = dac.tile([1, 1], f32)
                nc.vector.memset(dacc[:], 0.0)

                def dec_body(iv):
                    ch = dp.tile([fr, D], f32, tag="ch")
                    nc.sync.dma_start(ch[:], hsh_dram[ds(iv, fr), :])
                    pT = dps.tile([D, fr], f32, tag="dpT")
                    nc.tensor.transpose(pT[:], ch[:], ident[:fr, :fr])
                    hT = dp.tile([D, fr], f32, tag="hT")
                    nc.vector.tensor_copy(hT[:], pT[:])
                    pz = dps.tile([1, fr], f32, tag="pz")
                    nc.tensor.matmul(pz[:], decW_sb[:], hT[:], start=True, stop=True)
                    zz = dp.tile([1, fr], f32, tag="zz")
                    zs = dp.tile([1, 1], f32, tag="zs")
                    nc.scalar.activation(zz[:], pz[:], AT.Sigmoid,
                                         bias=decb_sb[:], accum_out=zs[:])
                    nc.vector.tensor_tensor(out=dacc[:], in0=dacc[:], in1=zs[:], op=OP.add)
                tc.For_i_unrolled(0, nsh, fr, dec_body, max_unroll=4)
                nc.sync.dma_start(outp[:, :], dacc[:])

    nc.finalize()
    return nc


def _get_nc(nsh, t_pad, fr, k, ncores):
    key = (nsh, t_pad, fr, k, ncores)
    if key not in _CACHE:
        _CACHE[key] = _build(nsh, t_pad, fr, k, ncores)
    return _CACHE[key]


def _prepare(x, edge_index, enc_W, enc_b, Wg, a_src, a_dst, bg, ln_w, ln_b,
             dec_W, dec_b):
    x = np.asarray(x, np.float32)
    n_full = x.shape[0]
    nsh = n_full // NC
    fr = next(f for f in range(min(P, nsh), 0, -1) if nsh % f == 0)
    ei = np.asarray(edge_index)
    loop = np.arange(n_full, dtype=ei.dtype)
    src = np.concatenate([ei[0], loop])
    dst = np.concatenate([ei[1], loop])
    cores, t_pad, k = _prep(src, dst, n_full, nsh)

    enc_b = np.asarray(enc_b, np.float32)
    Wg = np.asarray(Wg, np.float32)
    a2 = np.stack([np.asarray(a_src, np.float32),
                   np.asarray(a_dst, np.float32)], axis=2)  # [L, D, 2]
    bg_r = np.broadcast_to(np.asarray(bg, np.float32)[:, None, :], (L, P, D)).copy()
    lnw_r = np.broadcast_to(np.asarray(ln_w, np.float32)[:, None, :], (L, P, D)).copy()
    lnb_r = np.broadcast_to(np.asarray(ln_b, np.float32)[:, None, :], (L, P, D)).copy()
    encb_r = np.broadcast_to(enc_b[None, :], (P, D)).copy()
    decW_h = np.asarray(dec_W, np.float32).reshape(D, 1)
    decb_h = np.asarray(dec_b, np.float32).reshape(1, 1)

    nc = _get_nc(nsh, t_pad, fr, k, NC)
    in_maps = []
    for c in range(NC):
        m = {
            "xs": np.ascontiguousarray(x[c * nsh:(c + 1) * nsh]),
            "encW": np.ascontiguousarray(np.asarray(enc_W, np.float32)),
            "encb": encb_r, "WgP": Wg, "a2P": a2, "bgP": bg_r,
            "lnwP": lnw_r, "lnbP": lnb_r, "decW": decW_h, "decb": decb_h,
        }
        m.update(cores[c])
        in_maps.append(m)
    return nc, in_maps


def kernel(x, edge_index, enc_W, enc_b, Wg, a_src, a_dst, bg, ln_w, ln_b,
           dec_W, dec_b):
    from concourse.bass_utils import run_bass_kernel_spmd

    nc, in_maps = _prepare(x, edge_index, enc_W, enc_b, Wg, a_src, a_dst,
                           bg, ln_w, ln_b, dec_W, dec_b)
    res = run_bass_kernel_spmd(nc, in_maps, list(range(NC))).results
    total = np.float32(sum(float(res[c]["outp"][0, 0]) for c in range(NC)))
    return np.array([total], np.float32)


# revision 24
# speedup vs baseline: 668.0316x; 1.2793x over previous
"""3-layer GAT on Trainium2, 8 NeuronCores, full computation on device.

Sharding: nodes partitioned by dst ownership (nsh=12500/core). Edges are
dst-sorted and grouped into 128-edge tiles covering a FIXED range of K
consecutive dst nodes (K chosen at runtime so no K-node group exceeds
128 edges; K=5 for the reference graph). Fixed K makes every tile's
output rows linear in the tile index, so segment sums land in outbuf via
plain DMA writes — the only indirect DMA is the per-tile [128,1]-offset
row gather of hw_aug[src] (the hardware-validated DGE pattern).

Per layer each core: (1) recomputes full hw_aug = [h @ Wg | as] rows plus
an ad table (replicated compute instead of communicating hw), (2) edge
loop over tile groups: gather hw_aug[src] rows (as rides along as column
128), load the group's ad values with a linear partition_id-based slice,
build the per-(edge,slot) weight matrix M = exp(leaky_relu(as_e + ad_s))
* [slot(e)==s] with a handful of batched [128, G*K] vector ops, then per
tile two PE matmuls produce the K x (D+1) segment sums (features +
softmax denominator), (3) graph-LayerNorm via AllReduce of (sum, sumsq),
relu, residual, (4) AllGather of updated h shards. Softmax is max-free
(e stays in [-0.6, 1.8] at this data scale; exp cannot overflow).
Decoder computes per-core partial sigmoid-sums; host adds the 8 partials.
"""

import numpy as np

NC = 8
P = 128
D = 128
DA = 132          # hw_aug row: 128 feats + as + 3 pad (16B-aligned row)
L = 3
EPS = 1e-5
NEG = 0.2
CH = 512          # pre-pass node chunk
G = 16            # edge tiles per loop group

_CACHE = {}


def _choose_k(counts, nsh):
    for k in (6, 5, 4, 3, 2, 1):
        ok = True
        for c in range(NC):
            cc = counts[c * nsh:(c + 1) * nsh]
            gs = np.add.reduceat(cc, np.arange(0, nsh, k))
            if gs.max() > P:
                ok = False
                break
        if ok:
            return k
    raise AssertionError("node with degree > 128")


def _prep(src, dst, n_full, nsh):
    """Edge preprocessing -> per-core wrapped (src, slot) arrays + K."""
    perm = np.argsort(dst, kind="stable")
    src_s = np.ascontiguousarray(src[perm]).astype(np.int64)
    dst_s = np.ascontiguousarray(dst[perm]).astype(np.int64)
    starts = np.searchsorted(dst_s, np.arange(n_full + 1, dtype=np.int64), "left")
    counts = np.diff(starts)
    k = _choose_k(counts, nsh)
    ngrp = (nsh + k - 1) // k
    t_pad = ((ngrp + G - 1) // G) * G
    cores = []
    for c in range(NC):
        lo = c * nsh
        e0, e1 = starts[lo], starts[lo + nsh]
        ne = e1 - e0
        dstloc = dst_s[e0:e1] - lo
        g = dstloc // k
        s = (dstloc % k).astype(np.float32)
        grp_first = np.searchsorted(g, np.arange(ngrp))
        epos = g * P + (np.arange(ne) - grp_first[g])
        srcA = np.zeros(t_pad * P, np.int32)
        slotA = np.full(t_pad * P, float(k), np.float32)   # sentinel slot
        srcA[epos] = src_s[e0:e1]
        slotA[epos] = s
        cores.append({
            "srcA": np.ascontiguousarray(srcA.reshape(t_pad, P).T),
            "slotA": np.ascontiguousarray(slotA.reshape(t_pad, P).T),
        })
    return cores, t_pad, k


def _build(nsh, t_pad, fr, k, ncores):
    import concourse.bacc as bacc
    import concourse.tile as tile
    from concourse import mybir
    from concourse.bass import IndirectOffsetOnAxis, ds
    from concourse.masks import make_identity

    f32 = mybir.dt.float32
    i32 = mybir.dt.int32
    AT = mybir.ActivationFunctionType
    OP = mybir.AluOpType

    n_full = nsh * ncores
    n_pad = ((n_full + CH - 1) // CH) * CH
    while (ncores - 1) * nsh + k * t_pad + P > n_pad:
        n_pad += CH
    gk = G * k
    nrows = k * t_pad
    nd_inv = 1.0 / (float(n_full) * D)

    nc = bacc.Bacc()
    xs = nc.declare_dram_parameter("xs", [nsh, D], f32, isOutput=False)
    encW = nc.declare_dram_parameter("encW", [D, D], f32, isOutput=False)
    encb = nc.declare_dram_parameter("encb", [P, D], f32, isOutput=False)
    WgP = nc.declare_dram_parameter("WgP", [L, D, D], f32, isOutput=False)
    a2P = nc.declare_dram_parameter("a2P", [L, D, 2], f32, isOutput=False)
    bgP = nc.declare_dram_parameter("bgP", [L, P, D], f32, isOutput=False)
    lnwP = nc.declare_dram_parameter("lnwP", [L, P, D], f32, isOutput=False)
    lnbP = nc.declare_dram_parameter("lnbP", [L, P, D], f32, isOutput=False)
    decW = nc.declare_dram_parameter("decW", [D, 1], f32, isOutput=False)
    decb = nc.declare_dram_parameter("decb", [1, 1], f32, isOutput=False)
    srcA = nc.declare_dram_parameter("srcA", [P, t_pad], i32, isOutput=False)
    slotA = nc.declare_dram_parameter("slotA", [P, t_pad], f32, isOutput=False)
    outp = nc.declare_dram_parameter("outp", [1, 1], f32, isOutput=True)

    h_full = nc.dram_tensor("h_full", [n_pad, D], f32, addr_space="Shared")
    cc_in = nc.dram_tensor("cc_in", [nsh, D], f32)
    st_in = nc.dram_tensor("st_in", [1, 2], f32)
    st_out = nc.dram_tensor("st_out", [1, 2], f32, addr_space="Shared")
    hw_aug = nc.dram_tensor("hw_aug", [n_pad, DA], f32)
    ad_dram = nc.dram_tensor("ad_dram", [1, n_pad], f32)
    outbuf = nc.dram_tensor("outbuf", [nrows + P, D + 1], f32)
    conv_dram = nc.dram_tensor("conv_dram", [nsh, D], f32)
    hsh_dram = nc.dram_tensor("hsh_dram", [nsh, D], f32)
    rg = [list(range(ncores))]

    with tile.TileContext(nc) as tc:
        with (
            tc.tile_pool(name="cst", bufs=1) as cst,
            tc.tile_pool(name="wts", bufs=1) as wts,
            tc.tile_pool(name="sml", bufs=2) as sml,
        ):
            ident = cst.tile([P, P], f32)
            make_identity(nc, ident[:])
            iotaK = cst.tile([P, gk], f32)
            nc.gpsimd.iota(iotaK[:], pattern=[[0, G], [1, k]], base=0,
                           channel_multiplier=0,
                           allow_small_or_imprecise_dtypes=True)
            ones_col = cst.tile([P, 1], f32)
            nc.vector.memset(ones_col[:], 1.0)
            one_row = cst.tile([1, P], f32)
            nc.vector.memset(one_row[:], 1.0)
            if n_pad > n_full:
                zrow = cst.tile([P, D], f32)
                nc.vector.memset(zrow[:], 0.0)
                r = n_full
                while r < n_pad:
                    q = min(P, n_pad - r)
                    nc.sync.dma_start(h_full[r:r + q, :], zrow[:q, :])
                    r += q

            encW_sb = wts.tile([D, D], f32)
            nc.sync.dma_start(encW_sb[:], encW[:])
            encb_sb = wts.tile([P, D], f32)
            nc.sync.dma_start(encb_sb[:], encb[:])
            decW_sb = wts.tile([D, 1], f32)
            nc.sync.dma_start(decW_sb[:], decW[:])
            decb_sb = wts.tile([1, 1], f32)
            nc.sync.dma_start(decb_sb[:], decb[:])

            # ---------------- encoder ----------------
            with (
                tc.tile_pool(name="ep", bufs=3) as ep,
                tc.tile_pool(name="eps", bufs=2, space="PSUM") as eps,
            ):
                def enc_body(iv):
                    xt = ep.tile([fr, D], f32, tag="xt")
                    nc.sync.dma_start(xt[:], xs[ds(iv, fr), :])
                    pT = eps.tile([D, fr], f32, tag="pT")
                    nc.tensor.transpose(pT[:], xt[:], ident[:fr, :fr])
                    xT = ep.tile([D, fr], f32, tag="xT")
                    nc.vector.tensor_copy(xT[:], pT[:])
                    ph = eps.tile([fr, D], f32, tag="ph")
                    nc.tensor.matmul(ph[:], xT[:], encW_sb[:], start=True, stop=True)
                    h0 = ep.tile([fr, D], f32, tag="h0")
                    nc.vector.tensor_tensor(out=h0[:], in0=ph[:], in1=encb_sb[:fr, :], op=OP.add)
                    nc.sync.dma_start(cc_in[ds(iv, fr), :], h0[:])
                    nc.sync.dma_start(hsh_dram[ds(iv, fr), :], h0[:])
                tc.For_i_unrolled(0, nsh, fr, enc_body, max_unroll=4)

            nc.gpsimd.collective_compute(
                "AllGather", OP.bypass, replica_groups=rg,
                ins=[cc_in[:, :]], outs=[h_full[0:n_full, :]])

            # ---------------- layers ----------------
            for l in range(L):
                with tc.tile_pool(name=f"lw{l}", bufs=1) as lw:
                    Wg_sb = lw.tile([D, D], f32)
                    nc.sync.dma_start(Wg_sb[:], WgP[l, :, :])
                    a2_sb = lw.tile([D, 2], f32)
                    nc.sync.dma_start(a2_sb[:], a2P[l, :, :])
                    bg_sb = lw.tile([P, D], f32)
                    nc.sync.dma_start(bg_sb[:], bgP[l, :, :])
                    lnw_sb = lw.tile([P, D], f32)
                    nc.sync.dma_start(lnw_sb[:], lnwP[l, :, :])
                    lnb_sb = lw.tile([P, D], f32)
                    nc.sync.dma_start(lnb_sb[:], lnbP[l, :, :])

                    # ---- pre-pass: hw_aug rows + ad table, all nodes ----
                    with (
                        tc.tile_pool(name="pp", bufs=3) as pp,
                        tc.tile_pool(name="ppsA", bufs=2, space="PSUM") as ppsA,
                        tc.tile_pool(name="ppsB", bufs=2, space="PSUM") as ppsB,
                        tc.tile_pool(name="ppsC", bufs=1, space="PSUM") as ppsC,
                    ):
                        def pre_body(iv):
                            hch = pp.tile([P, 4, D], f32, tag="hch")
                            nc.sync.dma_start(
                                hch[:], h_full[ds(iv, CH), :].rearrange(
                                    "(a p) d -> p a d", p=P))
                            pT = ppsA.tile([P, CH], f32, tag="pT")
                            for b in range(4):
                                nc.tensor.transpose(
                                    pT[:, b * P:(b + 1) * P], hch[:, b, :], ident[:])
                            hT = pp.tile([P, CH], f32, tag="hT")
                            nc.vector.tensor_copy(hT[:], pT[:])
                            phw = ppsB.tile([P, CH], f32, tag="phw")
                            nc.tensor.matmul(phw[:], Wg_sb[:], hT[:], start=True, stop=True)
                            hwT = pp.tile([P, CH], f32, tag="hwT")
                            nc.vector.tensor_copy(hwT[:], phw[:])
                            pas = ppsC.tile([2, CH], f32, tag="pas")
                            nc.tensor.matmul(pas[:], a2_sb[:], hwT[:], start=True, stop=True)
                            phw2 = ppsA.tile([P, CH], f32, tag="phw2")
                            for b in range(4):
                                nc.tensor.transpose(
                                    phw2[:, b * P:(b + 1) * P],
                                    hwT[:, b * P:(b + 1) * P], ident[:])
                            asv = pp.tile([2, CH], f32, tag="asv")
                            nc.vector.tensor_copy(asv[:], pas[:])
                            pasT = ppsC.tile([P, 8], f32, tag="pasT")
                            for b in range(4):
                                nc.tensor.transpose(
                                    pasT[:, b * 2:(b + 1) * 2],
                                    asv[:, b * P:(b + 1) * P], ident[:2, :2])
                            stg = pp.tile([P, 4, DA], f32, tag="stg")
                            nc.vector.tensor_copy(
                                stg[:, :, 0:D], phw2[:].rearrange("p (a d) -> p a d", a=4))
                            nc.scalar.activation(
                                stg[:, :, D:D + 2],
                                pasT[:].rearrange("p (a t) -> p a t", a=4),
                                AT.Copy)
                            nc.vector.memset(stg[:, :, D + 2:DA], 0.0)
                            nc.sync.dma_start(
                                hw_aug[ds(iv, CH), :].rearrange("(a p) d -> p a d", p=P),
                                stg[:])
                            nc.sync.dma_start(ad_dram[0:1, ds(iv, CH)], asv[1:2, :])
                        tc.For_i_unrolled(0, n_pad, CH, pre_body, max_unroll=4)

                    # ---- edge loop ----
                    with (
                        tc.tile_pool(name="eb", bufs=3) as eb,
                        tc.tile_pool(name="ebR", bufs=2, space="PSUM") as ebR,
                        tc.tile_pool(name="ebo", bufs=4, space="PSUM") as ebo,
                    ):
                        pid_s = nc.sync.partition_id()
                        with tc.For_i(0, t_pad, G) as i0:
                            adr = eb.tile([1, P], f32, tag="adr")
                            nc.sync.dma_start(
                                adr[:], ad_dram[0:1, ds(pid_s * nsh + i0 * k, P)])
                            Rp = ebR.tile([P, gk], f32, tag="Rp")
                            nc.tensor.matmul(Rp[:], one_row[:], adr[:, 0:gk],
                                             start=True, stop=True)
                            six = eb.tile([P, G], i32, tag="six")
                            nc.sync.dma_start(six[:], srcA[:, ds(i0, G)])
                            slb = eb.tile([P, G, 1], f32, tag="slb")
                            nc.sync.dma_start(slb[:, :, 0], slotA[:, ds(i0, G)])
                            msgb = eb.tile([P, G, DA], f32, tag="msgb")
                            for j in range(G):
                                nc.gpsimd.indirect_dma_start(
                                    out=msgb[:, j, :], out_offset=None, in_=hw_aug[:],
                                    in_offset=IndirectOffsetOnAxis(
                                        ap=six[:, j:j + 1], axis=0))
                            Bt = eb.tile([P, G, k], f32, tag="Bt")
                            nc.vector.tensor_tensor(
                                out=Bt[:],
                                in0=Rp[:].rearrange("p (g s) -> p g s", g=G),
                                in1=msgb[:, :, D:D + 1].to_broadcast([P, G, k]),
                                op=OP.add)
                            lr = eb.tile([P, G, k], f32, tag="lr")
                            nc.vector.tensor_scalar_mul(lr[:], Bt[:], NEG)
                            nc.vector.tensor_tensor(out=lr[:], in0=lr[:], in1=Bt[:], op=OP.max)
                            M0 = eb.tile([P, G, k], f32, tag="M0")
                            nc.scalar.activation(M0[:], lr[:], AT.Exp)
                            Sel = eb.tile([P, G, k], f32, tag="Sel")
                            nc.vector.tensor_tensor(
                                out=Sel[:],
                                in0=slb[:].to_broadcast([P, G, k]),
                                in1=iotaK[:].rearrange("p (g s) -> p g s", g=G),
                                op=OP.is_equal)
                            nc.vector.tensor_tensor(out=Sel[:], in0=Sel[:], in1=M0[:], op=OP.mult)
                            for j in range(G):
                                po = ebo.tile([k, D + 1], f32, tag="po")
                                nc.tensor.matmul(po[:, 0:D], Sel[:, j, :],
                                                 msgb[:, j, 0:D], start=True, stop=True)
                                nc.tensor.matmul(po[:, D:D + 1], Sel[:, j, :],
                                                 ones_col[:], start=True, stop=True)
                                st = eb.tile([k, D + 1], f32, tag="st")
                                if j % 2 == 0:
                                    nc.vector.tensor_copy(st[:], po[:])
                                else:
                                    nc.scalar.activation(st[:], po[:], AT.Copy)
                                nc.sync.dma_start(
                                    outbuf[ds(i0 * k + j * k, k), :], st[:])

                    # ---- finalize: conv = msg/denom + bg ; stats ----
                    with (
                        tc.tile_pool(name="fp", bufs=3) as fp,
                        tc.tile_pool(name="facc", bufs=1) as facc,
                        tc.tile_pool(name="fps", bufs=2, space="PSUM") as fps,
                    ):
                        acc = facc.tile([fr, 2], f32)
                        nc.vector.memset(acc[:], 0.0)

                        def fin_body(iv):
                            ob = fp.tile([fr, D + 1], f32, tag="ob")
                            nc.sync.dma_start(ob[:], outbuf[ds(iv, fr), :])
                            rcp = fp.tile([fr, 1], f32, tag="rcp")
                            nc.vector.reciprocal(rcp[:], ob[:, D:D + 1])
                            cv = fp.tile([fr, D], f32, tag="cv")
                            nc.vector.tensor_tensor(
                                out=cv[:], in0=ob[:, 0:D],
                                in1=rcp[:].to_broadcast([fr, D]), op=OP.mult)
                            nc.vector.tensor_tensor(
                                out=cv[:], in0=cv[:], in1=bg_sb[:fr, :], op=OP.add)
                            s1 = fp.tile([fr, 1], f32, tag="s1")
                            nc.vector.tensor_reduce(
                                out=s1[:], in_=cv[:], axis=mybir.AxisListType.X, op=OP.add)
                            sqv = fp.tile([fr, D], f32, tag="sqv")
                            s2 = fp.tile([fr, 1], f32, tag="s2")
                            nc.scalar.activation(sqv[:], cv[:], AT.Square, accum_out=s2[:])
                            nc.vector.tensor_tensor(
                                out=acc[:, 0:1], in0=acc[:, 0:1], in1=s1[:], op=OP.add)
                            nc.vector.tensor_tensor(
                                out=acc[:, 1:2], in0=acc[:, 1:2], in1=s2[:], op=OP.add)
                            nc.sync.dma_start(conv_dram[ds(iv, fr), :], cv[:])
                        tc.For_i_unrolled(0, nsh, fr, fin_body, max_unroll=4)

                        pst = fps.tile([1, 2], f32)
                        nc.tensor.matmul(pst[:], ones_col[:fr, :], acc[:], start=True, stop=True)
                        stt = sml.tile([1, 2], f32, tag="stt")
                        nc.vector.tensor_copy(stt[:], pst[:])
                        nc.sync.dma_start(st_in[:, :], stt[:])

                    nc.gpsimd.collective_compute(
                        "AllReduce", OP.add, replica_groups=rg,
                        ins=[st_in[:, :]], outs=[st_out[:, :]])

                    # ---- stats -> scale/shift, apply LN + relu + residual ----
                    with (
                        tc.tile_pool(name="ap", bufs=3) as apl,
                        tc.tile_pool(name="aps", bufs=2, space="PSUM") as aps,
                    ):
                        sto = sml.tile([1, 2], f32, tag="sto")
                        nc.sync.dma_start(sto[:], st_out[:, :])
                        mn = sml.tile([1, 1], f32, tag="mn")
                        nc.vector.tensor_scalar_mul(mn[:], sto[:, 0:1], nd_inv)
                        ms = sml.tile([1, 1], f32, tag="ms")
                        nc.vector.tensor_scalar_mul(ms[:], sto[:, 1:2], nd_inv)
                        m2 = sml.tile([1, 1], f32, tag="m2")
                        nc.vector.tensor_tensor(out=m2[:], in0=mn[:], in1=mn[:], op=OP.mult)
                        vr = sml.tile([1, 1], f32, tag="vr")
                        nc.vector.tensor_tensor(out=vr[:], in0=ms[:], in1=m2[:], op=OP.subtract)
                        nc.vector.tensor_scalar_add(vr[:], vr[:], EPS)
                        sd = sml.tile([1, 1], f32, tag="sd")
                        nc.scalar.activation(sd[:], vr[:], AT.Sqrt)
                        rs = sml.tile([1, 1], f32, tag="rs")
                        nc.vector.reciprocal(rs[:], sd[:])
                        nmr = sml.tile([1, 1], f32, tag="nmr")
                        nc.vector.tensor_tensor(out=nmr[:], in0=mn[:], in1=rs[:], op=OP.mult)
                        nc.vector.tensor_scalar_mul(nmr[:], nmr[:], -1.0)
                        pk = sml.tile([1, 2], f32, tag="pk")
                        nc.vector.tensor_copy(pk[:, 0:1], rs[:])
                        nc.vector.tensor_copy(pk[:, 1:2], nmr[:])
                        pbc = aps.tile([P, 2], f32)
                        nc.tensor.matmul(pbc[:], one_row[:], pk[:], start=True, stop=True)
                        bc = sml.tile([P, 2], f32, tag="bc")
                        nc.vector.tensor_copy(bc[:], pbc[:])

                        last = (l == L - 1)

                        def app_body(iv):
                            cv = apl.tile([fr, D], f32, tag="acv")
                            nc.sync.dma_start(cv[:], conv_dram[ds(iv, fr), :])
                            tt = apl.tile([fr, D], f32, tag="att")
                            nc.vector.tensor_scalar(
                                out=tt[:], in0=cv[:], scalar1=bc[:fr, 0:1],
                                scalar2=bc[:fr, 1:2], op0=OP.mult, op1=OP.add)
                            nc.vector.tensor_tensor(
                                out=tt[:], in0=tt[:], in1=lnw_sb[:fr, :], op=OP.mult)
                            nc.vector.tensor_tensor(
                                out=tt[:], in0=tt[:], in1=lnb_sb[:fr, :], op=OP.add)
                            nc.vector.tensor_scalar_max(tt[:], tt[:], 0.0)
                            hin = apl.tile([fr, D], f32, tag="hin")
                            nc.sync.dma_start(hin[:], hsh_dram[ds(iv, fr), :])
                            nc.vector.tensor_tensor(
                                out=tt[:], in0=tt[:], in1=hin[:], op=OP.add)
                            nc.sync.dma_start(hsh_dram[ds(iv, fr), :], tt[:])
                            if not last:
                                nc.sync.dma_start(cc_in[ds(iv, fr), :], tt[:])
                        tc.For_i_unrolled(0, nsh, fr, app_body, max_unroll=4)

                    if l < L - 1:
                        nc.gpsimd.collective_compute(
                            "AllGather", OP.bypass, replica_groups=rg,
                            ins=[cc_in[:, :]], outs=[h_full[0:n_full, :]])

            # ---------------- decoder ----------------
            with (
                tc.tile_pool(name="dp", bufs=3) as dp,
                tc.tile_pool(name="dacc", bufs=1) as dac,
                tc.tile_pool(name="dps", bufs=2, space="PSUM") as dps,
            ):
                dacc = dac.tile([1, 1], f32)
                nc.vector.memset(dacc[:], 0.0)

                def dec_body(iv):
                    ch = dp.tile([fr, D], f32, tag="ch")
                    nc.sync.dma_start(ch[:], hsh_dram[ds(iv, fr), :])
                    pT = dps.tile([D, fr], f32, tag="dpT")
                    nc.tensor.transpose(pT[:], ch[:], ident[:fr, :fr])
                    hT = dp.tile([D, fr], f32, tag="hT")
                    nc.vector.tensor_copy(hT[:], pT[:])
                    pz = dps.tile([1, fr], f32, tag="pz")
                    nc.tensor.matmul(pz[:], decW_sb[:], hT[:], start=True, stop=True)
                    zz = dp.tile([1, fr], f32, tag="zz")
                    zs = dp.tile([1, 1], f32, tag="zs")
                    nc.scalar.activation(zz[:], pz[:], AT.Sigmoid,
                                         bias=decb_sb[:], accum_out=zs[:])
                    nc.vector.tensor_tensor(out=dacc[:], in0=dacc[:], in1=zs[:], op=OP.add)
                tc.For_i_unrolled(0, nsh, fr, dec_body, max_unroll=4)
                nc.sync.dma_start(outp[:, :], dacc[:])

    nc.finalize()
    return nc


def _get_nc(nsh, t_pad, fr, k, ncores):
    key = (nsh, t_pad, fr, k, ncores)
    if key not in _CACHE:
        _CACHE[key] = _build(nsh, t_pad, fr, k, ncores)
    return _CACHE[key]


def _prepare(x, edge_index, enc_W, enc_b, Wg, a_src, a_dst, bg, ln_w, ln_b,
             dec_W, dec_b):
    x = np.asarray(x, np.float32)
    n_full = x.shape[0]
    nsh = n_full // NC
    fr = next(f for f in range(min(P, nsh), 0, -1) if nsh % f == 0)
    ei = np.asarray(edge_index)
    loop = np.arange(n_full, dtype=ei.dtype)
    src = np.concatenate([ei[0], loop])
    dst = np.concatenate([ei[1], loop])
    cores, t_pad, k = _prep(src, dst, n_full, nsh)

    enc_b = np.asarray(enc_b, np.float32)
    Wg = np.asarray(Wg, np.float32)
    a2 = np.stack([np.asarray(a_src, np.float32),
                   np.asarray(a_dst, np.float32)], axis=2)  # [L, D, 2]
    bg_r = np.broadcast_to(np.asarray(bg, np.float32)[:, None, :], (L, P, D)).copy()
    lnw_r = np.broadcast_to(np.asarray(ln_w, np.float32)[:, None, :], (L, P, D)).copy()
    lnb_r = np.broadcast_to(np.asarray(ln_b, np.float32)[:, None, :], (L, P, D)).copy()
    encb_r = np.broadcast_to(enc_b[None, :], (P, D)).copy()
    decW_h = np.asarray(dec_W, np.float32).reshape(D, 1)
    decb_h = np.asarray(dec_b, np.float32).reshape(1, 1)

    nc = _get_nc(nsh, t_pad, fr, k, NC)
    in_maps = []
    for c in range(NC):
        m = {
            "xs": np.ascontiguousarray(x[c * nsh:(c + 1) * nsh]),
            "encW": np.ascontiguousarray(np.asarray(enc_W, np.float32)),
            "encb": encb_r, "WgP": Wg, "a2P": a2, "bgP": bg_r,
            "lnwP": lnw_r, "lnbP": lnb_r, "decW": decW_h, "decb": decb_h,
        }
        m.update(cores[c])
        in_maps.append(m)
    return nc, in_maps


def kernel(x, edge_index, enc_W, enc_b, Wg, a_src, a_dst, bg, ln_w, ln_b,
           dec_W, dec_b):
    from concourse.bass_utils import run_bass_kernel_spmd

    nc, in_maps = _prepare(x, edge_index, enc_W, enc_b, Wg, a_src, a_dst,
                           bg, ln_w, ln_b, dec_W, dec_b)
    res = run_bass_kernel_spmd(nc, in_maps, list(range(NC))).results
    total = np.float32(sum(float(res[c]["outp"][0, 0]) for c in range(NC)))
    return np.array([total], np.float32)


# revision 26
# speedup vs baseline: 686.1685x; 1.0271x over previous
"""3-layer GAT on Trainium2, 8 NeuronCores, full computation on device.

Sharding: nodes partitioned by dst ownership (nsh=12500/core). Edges are
dst-sorted and grouped into 128-edge tiles covering a FIXED range of K
consecutive dst nodes (K chosen at runtime so no K-node group exceeds
128 edges; K=5 for the reference graph). Fixed K makes every tile's
output rows linear in the tile index, so segment sums land in outbuf via
plain DMA writes — the only indirect DMA is the per-tile [128,1]-offset
row gather of hw_aug[src] (the hardware-validated DGE pattern).

Per layer each core: (1) recomputes full hw_aug = [h @ Wg | as] rows plus
an ad table (replicated compute instead of communicating hw), (2) edge
loop over tile groups: gather hw_aug[src] rows (as rides along as column
128), load the group's ad values with a linear partition_id-based slice,
build the per-(edge,slot) weight matrix M = exp(leaky_relu(as_e + ad_s))
* [slot(e)==s] with a handful of batched [128, G*K] vector ops, then per
tile two PE matmuls produce the K x (D+1) segment sums (features +
softmax denominator), (3) graph-LayerNorm via AllReduce of (sum, sumsq),
relu, residual, (4) AllGather of updated h shards. Softmax is max-free
(e stays in [-0.6, 1.8] at this data scale; exp cannot overflow).
Decoder computes per-core partial sigmoid-sums; host adds the 8 partials.
"""

import numpy as np

NC = 8
P = 128
D = 128
DA = 132          # hw_aug row: 128 feats + as + 3 pad (16B-aligned row)
L = 3
EPS = 1e-5
NEG = 0.2
CH = 512          # pre-pass node chunk
G = 32            # edge tiles per loop group

_CACHE = {}


def _choose_k(counts, nsh):
    for k in (6, 5, 4, 3, 2, 1):
        ok = True
        for c in range(NC):
            cc = counts[c * nsh:(c + 1) * nsh]
            gs = np.add.reduceat(cc, np.arange(0, nsh, k))
            if gs.max() > P:
                ok = False
                break
        if ok:
            return k
    raise AssertionError("node with degree > 128")


def _prep(src, dst, n_full, nsh):
    """Edge preprocessing -> per-core wrapped (src, slot) arrays + K."""
    perm = np.argsort(dst, kind="stable")
    src_s = np.ascontiguousarray(src[perm]).astype(np.int64)
    dst_s = np.ascontiguousarray(dst[perm]).astype(np.int64)
    starts = np.searchsorted(dst_s, np.arange(n_full + 1, dtype=np.int64), "left")
    counts = np.diff(starts)
    k = _choose_k(counts, nsh)
    ngrp = (nsh + k - 1) // k
    t_pad = ((ngrp + G - 1) // G) * G
    cores = []
    for c in range(NC):
        lo = c * nsh
        e0, e1 = starts[lo], starts[lo + nsh]
        ne = e1 - e0
        dstloc = dst_s[e0:e1] - lo
        g = dstloc // k
        s = (dstloc % k).astype(np.float32)
        grp_first = np.searchsorted(g, np.arange(ngrp))
        epos = g * P + (np.arange(ne) - grp_first[g])
        srcA = np.zeros(t_pad * P, np.int32)
        slotA = np.full(t_pad * P, float(k), np.float32)   # sentinel slot
        srcA[epos] = src_s[e0:e1]
        slotA[epos] = s
        cores.append({
            "srcA": np.ascontiguousarray(srcA.reshape(t_pad, P).T),
            "slotA": np.ascontiguousarray(slotA.reshape(t_pad, P).T),
        })
    return cores, t_pad, k


def _build(nsh, t_pad, fr, k, ncores):
    import concourse.bacc as bacc
    import concourse.tile as tile
    from concourse import mybir
    from concourse.bass import IndirectOffsetOnAxis, ds
    from concourse.masks import make_identity

    f32 = mybir.dt.float32
    i32 = mybir.dt.int32
    AT = mybir.ActivationFunctionType
    OP = mybir.AluOpType

    n_full = nsh * ncores
    n_pad = ((n_full + CH - 1) // CH) * CH
    while (ncores - 1) * nsh + k * t_pad + G * k > n_pad:
        n_pad += CH
    gk = G * k
    nrows = k * t_pad
    nd_inv = 1.0 / (float(n_full) * D)

    nc = bacc.Bacc()
    xs = nc.declare_dram_parameter("xs", [nsh, D], f32, isOutput=False)
    encW = nc.declare_dram_parameter("encW", [D, D], f32, isOutput=False)
    encb = nc.declare_dram_parameter("encb", [P, D], f32, isOutput=False)
    WgP = nc.declare_dram_parameter("WgP", [L, D, D], f32, isOutput=False)
    a2P = nc.declare_dram_parameter("a2P", [L, D, 2], f32, isOutput=False)
    bgP = nc.declare_dram_parameter("bgP", [L, P, D], f32, isOutput=False)
    lnwP = nc.declare_dram_parameter("lnwP", [L, P, D], f32, isOutput=False)
    lnbP = nc.declare_dram_parameter("lnbP", [L, P, D], f32, isOutput=False)
    decW = nc.declare_dram_parameter("decW", [D, 1], f32, isOutput=False)
    decb = nc.declare_dram_parameter("decb", [1, 1], f32, isOutput=False)
    srcA = nc.declare_dram_parameter("srcA", [P, t_pad], i32, isOutput=False)
    slotA = nc.declare_dram_parameter("slotA", [P, t_pad], f32, isOutput=False)
    outp = nc.declare_dram_parameter("outp", [1, 1], f32, isOutput=True)

    h_full = nc.dram_tensor("h_full", [n_pad, D], f32, addr_space="Shared")
    cc_in = nc.dram_tensor("cc_in", [nsh, D], f32)
    st_in = nc.dram_tensor("st_in", [1, 2], f32)
    st_out = nc.dram_tensor("st_out", [1, 2], f32, addr_space="Shared")
    hw_aug = nc.dram_tensor("hw_aug", [n_pad, DA], f32)
    ad_dram = nc.dram_tensor("ad_dram", [1, n_pad], f32)
    outbuf = nc.dram_tensor("outbuf", [nrows + P, D + 1], f32)
    conv_dram = nc.dram_tensor("conv_dram", [nsh, D], f32)
    hsh_dram = nc.dram_tensor("hsh_dram", [nsh, D], f32)
    rg = [list(range(ncores))]

    with tile.TileContext(nc) as tc:
        with (
            tc.tile_pool(name="cst", bufs=1) as cst,
            tc.tile_pool(name="wts", bufs=1) as wts,
            tc.tile_pool(name="sml", bufs=2) as sml,
        ):
            ident = cst.tile([P, P], f32)
            make_identity(nc, ident[:])
            iotaK = cst.tile([P, gk], f32)
            nc.gpsimd.iota(iotaK[:], pattern=[[0, G], [1, k]], base=0,
                           channel_multiplier=0,
                           allow_small_or_imprecise_dtypes=True)
            ones_col = cst.tile([P, 1], f32)
            nc.vector.memset(ones_col[:], 1.0)
            one_row = cst.tile([1, P], f32)
            nc.vector.memset(one_row[:], 1.0)
            if n_pad > n_full:
                zrow = cst.tile([P, D], f32)
                nc.vector.memset(zrow[:], 0.0)
                r = n_full
                while r < n_pad:
                    q = min(P, n_pad - r)
                    nc.sync.dma_start(h_full[r:r + q, :], zrow[:q, :])
                    r += q

            encW_sb = wts.tile([D, D], f32)
            nc.sync.dma_start(encW_sb[:], encW[:])
            encb_sb = wts.tile([P, D], f32)
            nc.sync.dma_start(encb_sb[:], encb[:])
            decW_sb = wts.tile([D, 1], f32)
            nc.sync.dma_start(decW_sb[:], decW[:])
            decb_sb = wts.tile([1, 1], f32)
            nc.sync.dma_start(decb_sb[:], decb[:])

            # ---------------- encoder ----------------
            with (
                tc.tile_pool(name="ep", bufs=3) as ep,
                tc.tile_pool(name="eps", bufs=2, space="PSUM") as eps,
            ):
                def enc_body(iv):
                    xt = ep.tile([fr, D], f32, tag="xt")
                    nc.sync.dma_start(xt[:], xs[ds(iv, fr), :])
                    pT = eps.tile([D, fr], f32, tag="pT")
                    nc.tensor.transpose(pT[:], xt[:], ident[:fr, :fr])
                    xT = ep.tile([D, fr], f32, tag="xT")
                    nc.vector.tensor_copy(xT[:], pT[:])
                    ph = eps.tile([fr, D], f32, tag="ph")
                    nc.tensor.matmul(ph[:], xT[:], encW_sb[:], start=True, stop=True)
                    h0 = ep.tile([fr, D], f32, tag="h0")
                    nc.vector.tensor_tensor(out=h0[:], in0=ph[:], in1=encb_sb[:fr, :], op=OP.add)
                    nc.sync.dma_start(cc_in[ds(iv, fr), :], h0[:])
                    nc.sync.dma_start(hsh_dram[ds(iv, fr), :], h0[:])
                tc.For_i_unrolled(0, nsh, fr, enc_body, max_unroll=4)

            nc.gpsimd.collective_compute(
                "AllGather", OP.bypass, replica_groups=rg,
                ins=[cc_in[:, :]], outs=[h_full[0:n_full, :]])

            # ---------------- layers ----------------
            for l in range(L):
                with tc.tile_pool(name=f"lw{l}", bufs=1) as lw:
                    Wg_sb = lw.tile([D, D], f32)
                    nc.sync.dma_start(Wg_sb[:], WgP[l, :, :])
                    a2_sb = lw.tile([D, 2], f32)
                    nc.sync.dma_start(a2_sb[:], a2P[l, :, :])
                    bg_sb = lw.tile([P, D], f32)
                    nc.sync.dma_start(bg_sb[:], bgP[l, :, :])
                    lnw_sb = lw.tile([P, D], f32)
                    nc.sync.dma_start(lnw_sb[:], lnwP[l, :, :])
                    lnb_sb = lw.tile([P, D], f32)
                    nc.sync.dma_start(lnb_sb[:], lnbP[l, :, :])

                    # ---- pre-pass: hw_aug rows + ad table, all nodes ----
                    with (
                        tc.tile_pool(name="pp", bufs=3) as pp,
                        tc.tile_pool(name="ppsA", bufs=2, space="PSUM") as ppsA,
                        tc.tile_pool(name="ppsB", bufs=2, space="PSUM") as ppsB,
                        tc.tile_pool(name="ppsC", bufs=1, space="PSUM") as ppsC,
                    ):
                        def pre_body(iv):
                            hch = pp.tile([P, 4, D], f32, tag="hch")
                            nc.sync.dma_start(
                                hch[:], h_full[ds(iv, CH), :].rearrange(
                                    "(a p) d -> p a d", p=P))
                            pT = ppsA.tile([P, CH], f32, tag="pT")
                            for b in range(4):
                                nc.tensor.transpose(
                                    pT[:, b * P:(b + 1) * P], hch[:, b, :], ident[:])
                            hT = pp.tile([P, CH], f32, tag="hT")
                            nc.vector.tensor_copy(hT[:], pT[:])
                            phw = ppsB.tile([P, CH], f32, tag="phw")
                            nc.tensor.matmul(phw[:], Wg_sb[:], hT[:], start=True, stop=True)
                            hwT = pp.tile([P, CH], f32, tag="hwT")
                            nc.vector.tensor_copy(hwT[:], phw[:])
                            pas = ppsC.tile([2, CH], f32, tag="pas")
                            nc.tensor.matmul(pas[:], a2_sb[:], hwT[:], start=True, stop=True)
                            phw2 = ppsA.tile([P, CH], f32, tag="phw2")
                            for b in range(4):
                                nc.tensor.transpose(
                                    phw2[:, b * P:(b + 1) * P],
                                    hwT[:, b * P:(b + 1) * P], ident[:])
                            asv = pp.tile([2, CH], f32, tag="asv")
                            nc.vector.tensor_copy(asv[:], pas[:])
                            pasT = ppsC.tile([P, 8], f32, tag="pasT")
                            for b in range(4):
                                nc.tensor.transpose(
                                    pasT[:, b * 2:(b + 1) * 2],
                                    asv[:, b * P:(b + 1) * P], ident[:2, :2])
                            stg = pp.tile([P, 4, DA], f32, tag="stg")
                            nc.vector.tensor_copy(
                                stg[:, :, 0:D], phw2[:].rearrange("p (a d) -> p a d", a=4))
                            nc.scalar.activation(
                                stg[:, :, D:D + 2],
                                pasT[:].rearrange("p (a t) -> p a t", a=4),
                                AT.Copy)
                            nc.vector.memset(stg[:, :, D + 2:DA], 0.0)
                            nc.sync.dma_start(
                                hw_aug[ds(iv, CH), :].rearrange("(a p) d -> p a d", p=P),
                                stg[:])
                            nc.sync.dma_start(ad_dram[0:1, ds(iv, CH)], asv[1:2, :])
                        tc.For_i_unrolled(0, n_pad, CH, pre_body, max_unroll=4)

                    # ---- edge loop ----
                    with (
                        tc.tile_pool(name="eb", bufs=4) as eb,
                        tc.tile_pool(name="ebR", bufs=2, space="PSUM") as ebR,
                        tc.tile_pool(name="ebo", bufs=4, space="PSUM") as ebo,
                    ):
                        pid_s = nc.sync.partition_id()
                        with tc.For_i(0, t_pad, G, staggered_reset=True) as i0:
                            adr = eb.tile([1, gk], f32, tag="adr")
                            nc.sync.dma_start(
                                adr[:], ad_dram[0:1, ds(pid_s * nsh + i0 * k, gk)])
                            Rp = ebR.tile([P, gk], f32, tag="Rp")
                            nc.tensor.matmul(Rp[:], one_row[:], adr[:, 0:gk],
                                             start=True, stop=True)
                            six = eb.tile([P, G], i32, tag="six")
                            nc.sync.dma_start(six[:], srcA[:, ds(i0, G)])
                            slb = eb.tile([P, G, 1], f32, tag="slb")
                            nc.sync.dma_start(slb[:, :, 0], slotA[:, ds(i0, G)])
                            msgb = eb.tile([P, G, DA], f32, tag="msgb")
                            for j in range(G):
                                nc.gpsimd.indirect_dma_start(
                                    out=msgb[:, j, :], out_offset=None, in_=hw_aug[:],
                                    in_offset=IndirectOffsetOnAxis(
                                        ap=six[:, j:j + 1], axis=0))
                            Bt = eb.tile([P, G, k], f32, tag="Bt")
                            nc.vector.tensor_tensor(
                                out=Bt[:],
                                in0=Rp[:].rearrange("p (g s) -> p g s", g=G),
                                in1=msgb[:, :, D:D + 1].to_broadcast([P, G, k]),
                                op=OP.add)
                            lr = eb.tile([P, G, k], f32, tag="lr")
                            nc.vector.tensor_scalar_mul(lr[:], Bt[:], NEG)
                            nc.vector.tensor_tensor(out=lr[:], in0=lr[:], in1=Bt[:], op=OP.max)
                            M0 = eb.tile([P, G, k], f32, tag="M0")
                            nc.scalar.activation(M0[:], lr[:], AT.Exp)
                            Sel = eb.tile([P, G, k], f32, tag="Sel")
                            nc.vector.tensor_tensor(
                                out=Sel[:],
                                in0=slb[:].to_broadcast([P, G, k]),
                                in1=iotaK[:].rearrange("p (g s) -> p g s", g=G),
                                op=OP.is_equal)
                            nc.vector.tensor_tensor(out=Sel[:], in0=Sel[:], in1=M0[:], op=OP.mult)
                            for j in range(G):
                                po = ebo.tile([k, D + 1], f32, tag="po")
                                nc.tensor.matmul(po[:, 0:D], Sel[:, j, :],
                                                 msgb[:, j, 0:D], start=True, stop=True)
                                nc.tensor.matmul(po[:, D:D + 1], Sel[:, j, :],
                                                 ones_col[:], start=True, stop=True)
                                st = eb.tile([k, D + 1], f32, tag="st")
                                if j % 2 == 0:
                                    nc.vector.tensor_copy(st[:], po[:])
                                else:
                                    nc.scalar.activation(st[:], po[:], AT.Copy)
                                nc.sync.dma_start(
                                    outbuf[ds(i0 * k + j * k, k), :], st[:])

                    # ---- finalize: conv = msg/denom + bg ; stats ----
                    with (
                        tc.tile_pool(name="fp", bufs=3) as fp,
                        tc.tile_pool(name="facc", bufs=1) as facc,
                        tc.tile_pool(name="fps", bufs=2, space="PSUM") as fps,
                    ):
                        acc = facc.tile([fr, 2], f32)
                        nc.vector.memset(acc[:], 0.0)

                        def fin_body(iv):
                            ob = fp.tile([fr, D + 1], f32, tag="ob")
                            nc.sync.dma_start(ob[:], outbuf[ds(iv, fr), :])
                            rcp = fp.tile([fr, 1], f32, tag="rcp")
                            nc.vector.reciprocal(rcp[:], ob[:, D:D + 1])
                            cv = fp.tile([fr, D], f32, tag="cv")
                            nc.vector.tensor_tensor(
                                out=cv[:], in0=ob[:, 0:D],
                                in1=rcp[:].to_broadcast([fr, D]), op=OP.mult)
                            nc.vector.tensor_tensor(
                                out=cv[:], in0=cv[:], in1=bg_sb[:fr, :], op=OP.add)
                            s1 = fp.tile([fr, 1], f32, tag="s1")
                            nc.vector.tensor_reduce(
                                out=s1[:], in_=cv[:], axis=mybir.AxisListType.X, op=OP.add)
                            sqv = fp.tile([fr, D], f32, tag="sqv")
                            s2 = fp.tile([fr, 1], f32, tag="s2")
                            nc.scalar.activation(sqv[:], cv[:], AT.Square, accum_out=s2[:])
                            nc.vector.tensor_tensor(
                                out=acc[:, 0:1], in0=acc[:, 0:1], in1=s1[:], op=OP.add)
                            nc.vector.tensor_tensor(
                                out=acc[:, 1:2], in0=acc[:, 1:2], in1=s2[:], op=OP.add)
                            nc.sync.dma_start(conv_dram[ds(iv, fr), :], cv[:])
                        tc.For_i_unrolled(0, nsh, fr, fin_body, max_unroll=4)

                        pst = fps.tile([1, 2], f32)
                        nc.tensor.matmul(pst[:], ones_col[:fr, :], acc[:], start=True, stop=True)
                        stt = sml.tile([1, 2], f32, tag="stt")
                        nc.vector.tensor_copy(stt[:], pst[:])
                        nc.sync.dma_start(st_in[:, :], stt[:])

                    nc.gpsimd.collective_compute(
                        "AllReduce", OP.add, replica_groups=rg,
                        ins=[st_in[:, :]], outs=[st_out[:, :]])

                    # ---- stats -> scale/shift, apply LN + relu + residual ----
                    with (
                        tc.tile_pool(name="ap", bufs=3) as apl,
                        tc.tile_pool(name="aps", bufs=2, space="PSUM") as aps,
                    ):
                        sto = sml.tile([1, 2], f32, tag="sto")
                        nc.sync.dma_start(sto[:], st_out[:, :])
                        mn = sml.tile([1, 1], f32, tag="mn")
                        nc.vector.tensor_scalar_mul(mn[:], sto[:, 0:1], nd_inv)
                        ms = sml.tile([1, 1], f32, tag="ms")
                        nc.vector.tensor_scalar_mul(ms[:], sto[:, 1:2], nd_inv)
                        m2 = sml.tile([1, 1], f32, tag="m2")
                        nc.vector.tensor_tensor(out=m2[:], in0=mn[:], in1=mn[:], op=OP.mult)
                        vr = sml.tile([1, 1], f32, tag="vr")
                        nc.vector.tensor_tensor(out=vr[:], in0=ms[:], in1=m2[:], op=OP.subtract)
                        nc.vector.tensor_scalar_add(vr[:], vr[:], EPS)
                        sd = sml.tile([1, 1], f32, tag="sd")
                        nc.scalar.activation(sd[:], vr[:], AT.Sqrt)
                        rs = sml.tile([1, 1], f32, tag="rs")
                        nc.vector.reciprocal(rs[:], sd[:])
                        nmr = sml.tile([1, 1], f32, tag="nmr")
                        nc.vector.tensor_tensor(out=nmr[:], in0=mn[:], in1=rs[:], op=OP.mult)
                        nc.vector.tensor_scalar_mul(nmr[:], nmr[:], -1.0)
                        pk = sml.tile([1, 2], f32, tag="pk")
                        nc.vector.tensor_copy(pk[:, 0:1], rs[:])
                        nc.vector.tensor_copy(pk[:, 1:2], nmr[:])
                        pbc = aps.tile([P, 2], f32)
                        nc.tensor.matmul(pbc[:], one_row[:], pk[:], start=True, stop=True)
                        bc = sml.tile([P, 2], f32, tag="bc")
                        nc.vector.tensor_copy(bc[:], pbc[:])

                        last = (l == L - 1)

                        def app_body(iv):
                            cv = apl.tile([fr, D], f32, tag="acv")
                            nc.sync.dma_start(cv[:], conv_dram[ds(iv, fr), :])
                            tt = apl.tile([fr, D], f32, tag="att")
                            nc.vector.tensor_scalar(
                                out=tt[:], in0=cv[:], scalar1=bc[:fr, 0:1],
                                scalar2=bc[:fr, 1:2], op0=OP.mult, op1=OP.add)
                            nc.vector.tensor_tensor(
                                out=tt[:], in0=tt[:], in1=lnw_sb[:fr, :], op=OP.mult)
                            nc.vector.tensor_tensor(
                                out=tt[:], in0=tt[:], in1=lnb_sb[:fr, :], op=OP.add)
                            nc.vector.tensor_scalar_max(tt[:], tt[:], 0.0)
                            hin = apl.tile([fr, D], f32, tag="hin")
                            nc.sync.dma_start(hin[:], hsh_dram[ds(iv, fr), :])
                            nc.vector.tensor_tensor(
                                out=tt[:], in0=tt[:], in1=hin[:], op=OP.add)
                            nc.sync.dma_start(hsh_dram[ds(iv, fr), :], tt[:])
                            if not last:
                                nc.sync.dma_start(cc_in[ds(iv, fr), :], tt[:])
                        tc.For_i_unrolled(0, nsh, fr, app_body, max_unroll=4)

                    if l < L - 1:
                        nc.gpsimd.collective_compute(
                            "AllGather", OP.bypass, replica_groups=rg,
                            ins=[cc_in[:, :]], outs=[h_full[0:n_full, :]])

            # ---------------- decoder ----------------
            with (
                tc.tile_pool(name="dp", bufs=3) as dp,
                tc.tile_pool(name="dacc", bufs=1) as dac,
                tc.tile_pool(name="dps", bufs=2, space="PSUM") as dps,
            ):
                dacc = dac.tile([1, 1], f32)
                nc.vector.memset(dacc[:], 0.0)

                def dec_body(iv):
                    ch = dp.tile([fr, D], f32, tag="ch")
                    nc.sync.dma_start(ch[:], hsh_dram[ds(iv, fr), :])
                    pT = dps.tile([D, fr], f32, tag="dpT")
                    nc.tensor.transpose(pT[:], ch[:], ident[:fr, :fr])
                    hT = dp.tile([D, fr], f32, tag="hT")
                    nc.vector.tensor_copy(hT[:], pT[:])
                    pz = dps.tile([1, fr], f32, tag="pz")
                    nc.tensor.matmul(pz[:], decW_sb[:], hT[:], start=True, stop=True)
                    zz = dp.tile([1, fr], f32, tag="zz")
                    zs = dp.tile([1, 1], f32, tag="zs")
                    nc.scalar.activation(zz[:], pz[:], AT.Sigmoid,
                                         bias=decb_sb[:], accum_out=zs[:])
                    nc.vector.tensor_tensor(out=dacc[:], in0=dacc[:], in1=zs[:], op=OP.add)
                tc.For_i_unrolled(0, nsh, fr, dec_body, max_unroll=4)
                nc.sync.dma_start(outp[:, :], dacc[:])

    nc.finalize()
    return nc


def _get_nc(nsh, t_pad, fr, k, ncores):
    key = (nsh, t_pad, fr, k, ncores)
    if key not in _CACHE:
        _CACHE[key] = _build(nsh, t_pad, fr, k, ncores)
    return _CACHE[key]


def _prepare(x, edge_index, enc_W, enc_b, Wg, a_src, a_dst, bg, ln_w, ln_b,
             dec_W, dec_b):
    x = np.asarray(x, np.float32)
    n_full = x.shape[0]
    nsh = n_full // NC
    fr = next(f for f in range(min(P, nsh), 0, -1) if nsh % f == 0)
    ei = np.asarray(edge_index)
    loop = np.arange(n_full, dtype=ei.dtype)
    src = np.concatenate([ei[0], loop])
    dst = np.concatenate([ei[1], loop])
    cores, t_pad, k = _prep(src, dst, n_full, nsh)

    enc_b = np.asarray(enc_b, np.float32)
    Wg = np.asarray(Wg, np.float32)
    a2 = np.stack([np.asarray(a_src, np.float32),
                   np.asarray(a_dst, np.float32)], axis=2)  # [L, D, 2]
    bg_r = np.broadcast_to(np.asarray(bg, np.float32)[:, None, :], (L, P, D)).copy()
    lnw_r = np.broadcast_to(np.asarray(ln_w, np.float32)[:, None, :], (L, P, D)).copy()
    lnb_r = np.broadcast_to(np.asarray(ln_b, np.float32)[:, None, :], (L, P, D)).copy()
    encb_r = np.broadcast_to(enc_b[None, :], (P, D)).copy()
    decW_h = np.asarray(dec_W, np.float32).reshape(D, 1)
    decb_h = np.asarray(dec_b, np.float32).reshape(1, 1)

    nc = _get_nc(nsh, t_pad, fr, k, NC)
    in_maps = []
    for c in range(NC):
        m = {
            "xs": np.ascontiguousarray(x[c * nsh:(c + 1) * nsh]),
            "encW": np.ascontiguousarray(np.asarray(enc_W, np.float32)),
            "encb": encb_r, "WgP": Wg, "a2P": a2, "bgP": bg_r,
            "lnwP": lnw_r, "lnbP": lnb_r, "decW": decW_h, "decb": decb_h,
        }
        m.update(cores[c])
        in_maps.append(m)
    return nc, in_maps


def kernel(x, edge_index, enc_W, enc_b, Wg, a_src, a_dst, bg, ln_w, ln_b,
           dec_W, dec_b):
    from concourse.bass_utils import run_bass_kernel_spmd

    nc, in_maps = _prepare(x, edge_index, enc_W, enc_b, Wg, a_src, a_dst,
                           bg, ln_w, ln_b, dec_W, dec_b)
    res = run_bass_kernel_spmd(nc, in_maps, list(range(NC))).results
    total = np.float32(sum(float(res[c]["outp"][0, 0]) for c in range(NC)))
    return np.array([total], np.float32)
